# revision 20
# baseline (speedup 1.0000x reference)
"""Trainium2 Bass kernel for nn_DiracGraphConv (GNN edge-softmax message passing).

Strategy (8 NeuronCores, SPMD, no collectives):
  - Shard edges by DESTINATION node range: core k owns nodes
    [k*12500, (k+1)*12500) and processes exactly the edges whose row
    (destination) falls in that range. Segment-sums for a node happen
    entirely on its owner core, so per-core results are disjoint node
    slabs and the full output is a host-side concatenation.
  - Within a core, edges are bucketed by col//25000 into 4 groups so
    gather indices fit int16 (dma_gather/dma_scatter_add contract).
  - The core's z slab is L2-normalized once on device (zh table), so the
    per-edge cosine needs only num = zh[row]&middot;z[col] and |z[col]|:
    corr = num / max(|z_col|, eps). exp shift constant is |alpha|
    (softmax shift invariance; bias_edge cancels).
  - Per gather-chunk: dma_gather zh[row] (row-local slab) and zx[col]
    (combined [z | x] 512-byte rows), compute logits and exp on DVE/ACT
    (exp lands directly in the message's 65th column), then
    dma_scatter_add the 65-wide message [e * x[col], e] into a per-core
    DRAM accumulator.
  - HW dma_scatter_add races on duplicate indices within an instruction
    (and across concurrently-flying instructions) — verified on HW.
    Countermeasures:
    (a) the host deals each (core,group,row)'s edges round-robin across
        scatter sub-chunks, so every scatter instruction carries unique
        row indices (pad tokens all hit a junk row; races there are
        harmless);
    (b) scatter instructions rotate across n_accums accumulator buffers;
        scatters on the same buffer are WAW-serialized by Tile sems, so
        same-row descriptors from different instructions are never in
        flight together. Final phase sums the buffers.
  - Final phase (batched 4 node-tiles per iteration):
    out = (num / (denom + eps)) @ W^T + b via PE transpose + matmul with
    [W^T; b] and an appended ones-column.
"""

import sys

sys.path.insert(0, "/opt/trn_rl_repo")

import dataclasses
from dataclasses import dataclass

import numpy as np

from concourse import bacc, bass, mybir, tile
from concourse.library_config import mlp as MLP_LIB
from concourse.masks import make_identity

P = 128
F32 = mybir.dt.float32
I16 = mybir.dt.int16
EPS_DENOM = 1e-9
EPS_NORM = 1e-9


@dataclass(frozen=True)
class Cfg:
    n_cores: int = 8
    n_nodes: int = 100000
    d: int = 64
    nodes_per_core: int = 12500
    col_groups: int = 4
    col_group_size: int = 25000
    # SWDGE carveout fits scratch/16 descriptors per instruction (16B/desc) —
    # every dma_gather/dma_scatter_add must stay below that.
    tokens_per_group: int = 52224  # multiple of scatter_b (auto-grown if needed)
    gather_b: int = 768  # max tokens per gather/compute chunk (<1024 descs)
    scatter_b: int = 768  # tokens per scatter instruction (unique rows)
    n_accums: int = 4
    swdge_queues: int = 2
    dma_scratch: int = 16384
    gbufs: int = 6
    xc1_on_dve: bool = False

    @property
    def acc_rows(self) -> int:
        # accumulator rows: nodes_per_core real + 1 junk row, padded to 128
        return ((self.nodes_per_core + 1 + P - 1) // P) * P

    @property
    def junk_row(self) -> int:
        return self.nodes_per_core

    @property
    def n_scatter_chunks(self) -> int:
        return self.tokens_per_group // self.scatter_b

    def gather_chunks(self):
        sizes = []
        t = self.tokens_per_group
        while t > 0:
            b = min(self.gather_b, t)
            sizes.append(b)
            t -= b
        assert all(s % self.scatter_b == 0 for s in sizes)
        return sizes


FULL = Cfg()


def build_program(cfg: Cfg, alpha: float, repeat: int = 1):
    """One SPMD program for all cores. Inputs (per core):
    zrow [acc_rows, d] f32   core's raw z slab
    zx   [col_groups*col_group_size, 2d] f32   full [z | x] table
    ridx [col_groups, 128, tokens_per_group//16] i16
    cidx [col_groups, 128, tokens_per_group//16] i16
    wb   [d+1, d] f32  ([W^T; b])
    Output: out [acc_rows, d] f32 (rows >= nodes_per_core are garbage)
    """
    D = cfg.d
    DD = 2 * D
    TG16 = cfg.tokens_per_group // 16
    SB = cfg.scatter_b

    nc = bacc.Bacc(
        "TRN2", target_bir_lowering=False, debug=False,
        num_swdge_queues=cfg.swdge_queues, dynamic_dma_scratch_size=cfg.dma_scratch,
    )

    zrow = nc.dram_tensor("zrow", [cfg.acc_rows, D], F32, kind="ExternalInput").ap()
    zxg = [
        nc.dram_tensor(f"zx{g}", [cfg.col_group_size, DD], F32, kind="ExternalInput").ap()
        for g in range(cfg.col_groups)
    ]
    ridx = nc.dram_tensor(
        "ridx", [cfg.col_groups, P, TG16], I16, kind="ExternalInput"
    ).ap()
    cidx = nc.dram_tensor(
        "cidx", [cfg.col_groups, P, TG16], I16, kind="ExternalInput"
    ).ap()
    wb = nc.dram_tensor("wb", [D + 1, D], F32, kind="ExternalInput").ap()
    out = nc.dram_tensor("out", [cfg.acc_rows, D], F32, kind="ExternalOutput").ap()

    zh = nc.dram_tensor("zh", [cfg.acc_rows, D], F32).ap()
    accums = [
        nc.dram_tensor(f"accum{s}", [cfg.acc_rows, DD], F32).ap()
        for s in range(cfg.n_accums)
    ]

    with tile.TileContext(nc) as tc:
        with (
            tc.tile_pool(name="const", bufs=1) as cpool,
            tc.tile_pool(name="idx", bufs=3) as ipool,
            tc.tile_pool(name="gath", bufs=2) as gpool,
            tc.tile_pool(name="work", bufs=2) as wpool,
            tc.tile_pool(name="smal", bufs=3) as spool,
            tc.tile_pool(name="fin", bufs=2) as fpool,
            tc.tile_pool(name="psum", bufs=2, space="PSUM") as ppool,
        ):
            # ---- constants ----
            nc.gpsimd.load_library(MLP_LIB)
            cb = cpool.tile([P, 1], F32, tag="cb")
            nc.vector.memset(cb[:], -abs(float(alpha)))
            ident = cpool.tile([P, P], F32, tag="ident")
            make_identity(nc, ident[:])
            wbs = cpool.tile([D + 1, D], F32, tag="wbs")
            nc.sync.dma_start(out=wbs[:], in_=wb[:, :])

            # ---- normalize the row slab: zh = zrow / max(|zrow|, eps) ----
            r0 = 0
            while r0 < cfg.acc_rows:
                j = min(8, (cfg.acc_rows - r0) // P)
                rows = slice(r0, r0 + j * P)
                zt_in = gpool.tile([P, 8, D], F32, tag="zi")
                nc.sync.dma_start(
                    out=zt_in[:, :j, :],
                    in_=zrow[rows, :].rearrange("(p a) d -> p a d", p=P),
                )
                sq = wpool.tile([P, 8, D], F32, tag="prod")
                nc.vector.tensor_tensor(
                    out=sq[:, :j, :], in0=zt_in[:, :j, :], in1=zt_in[:, :j, :],
                    op=mybir.AluOpType.mult,
                )
                ns = spool.tile([P, 8], F32, tag="ns")
                nc.vector.tensor_reduce(
                    out=ns[:, :j], in_=sq[:, :j, :], axis=mybir.AxisListType.X,
                    op=mybir.AluOpType.add,
                )
                nc.vector.tensor_scalar_max(ns[:, :j], ns[:, :j], 1e-18)
                nc.scalar.sqrt(out=ns[:, :j], in_=ns[:, :j])
                rr = spool.tile([P, 8], F32, tag="nr")
                nc.vector.reciprocal(out=rr[:, :j], in_=ns[:, :j])
                zo = gpool.tile([P, 8, D], F32, tag="gj")
                nc.vector.tensor_tensor(
                    out=zo[:, :j, :], in0=zt_in[:, :j, :],
                    in1=rr[:, :j].to_broadcast([P, j, D]), op=mybir.AluOpType.mult,
                )
                nc.sync.dma_start(
                    out=zh[rows, :].rearrange("(p a) d -> p a d", p=P),
                    in_=zo[:, :j, :],
                )
                r0 += j * P

            # ---- zero the accumulators ----
            acc_t = cfg.acc_rows // P
            zt = cpool.tile([P, 8 * DD], F32, tag="zt")
            nc.vector.memset(zt[:], 0.0)
            for acc in accums:
                acc_v = acc.rearrange("(t p) d -> p t d", p=P)
                for t0 in range(0, acc_t, 8):
                    nt = min(8, acc_t - t0)
                    nc.sync.dma_start(
                        out=acc_v[:, t0 : t0 + nt, :],
                        in_=zt[:, : nt * DD].rearrange("p (t d) -> p t d", d=DD),
                    )

            # ---- edge phase ----
            sc_counter = 0
            ci = 0
            for g in range(cfg.col_groups * repeat):
                g = g % cfg.col_groups
                zx_win = zxg[g][:, :]
                c0 = 0
                for b in cfg.gather_chunks():
                    nb = b // P
                    q = ci % cfg.swdge_queues
                    ci += 1
                    s16 = slice(c0 // 16, (c0 + b) // 16)
                    rt = ipool.tile([P, cfg.gather_b // 16], I16, tag="rt")
                    ct = ipool.tile([P, cfg.gather_b // 16], I16, tag="ct")
                    nc.sync.dma_start(out=rt[:, : b // 16], in_=ridx[g, :, s16])
                    nc.sync.dma_start(out=ct[:, : b // 16], in_=cidx[g, :, s16])

                    zi = gpool.tile([P, cfg.gather_b // P, D], F32, tag="zi")
                    gj = gpool.tile([P, cfg.gather_b // P, DD], F32, tag="gj")
                    nc.gpsimd.dma_gather(
                        zi[:, :nb, :], zh[:, :], rt[:, : b // 16], b, b, D,
                        queue_num=q,
                    )
                    nc.gpsimd.dma_gather(
                        gj[:, :nb, :], zx_win, ct[:, : b // 16], b, b, DD,
                        queue_num=q,
                    )
                    gjz = gj[:, :nb, 0:D]
                    gjx = gj[:, :nb, D:DD]

                    prod = wpool.tile([P, cfg.gather_b // P, D], F32, tag="prod")
                    num = spool.tile([P, cfg.gather_b // P], F32, tag="num")
                    nj = spool.tile([P, cfg.gather_b // P], F32, tag="nj")
                    nc.vector.tensor_tensor(
                        out=prod[:, :nb, :], in0=zi[:, :nb, :], in1=gjz,
                        op=mybir.AluOpType.mult,
                    )
                    nc.vector.tensor_reduce(
                        out=num[:, :nb], in_=prod[:, :nb, :],
                        axis=mybir.AxisListType.X, op=mybir.AluOpType.add,
                    )
                    nc.vector.tensor_tensor(
                        out=prod[:, :nb, :], in0=gjz, in1=gjz,
                        op=mybir.AluOpType.mult,
                    )
                    nc.vector.tensor_reduce(
                        out=nj[:, :nb], in_=prod[:, :nb, :],
                        axis=mybir.AxisListType.X, op=mybir.AluOpType.add,
                    )
                    nc.vector.tensor_scalar_max(nj[:, :nb], nj[:, :nb], 1e-18)
                    nc.scalar.sqrt(out=nj[:, :nb], in_=nj[:, :nb])
                    rcp = spool.tile([P, cfg.gather_b // P], F32, tag="rcp")
                    nc.vector.reciprocal(out=rcp[:, :nb], in_=nj[:, :nb])
                    corr = spool.tile([P, cfg.gather_b // P], F32, tag="corr")
                    nc.vector.tensor_tensor(
                        out=corr[:, :nb], in0=num[:, :nb], in1=rcp[:, :nb],
                        op=mybir.AluOpType.mult,
                    )
                    msg = wpool.tile([P, cfg.gather_b // P, D + 1], F32, tag="msg")
                    # exp lands directly in the message's denom column
                    nc.scalar.activation(
                        out=msg[:, :nb, D : D + 1], in_=corr[:, :nb],
                        func=mybir.ActivationFunctionType.Exp,
                        bias=cb[:], scale=float(alpha),
                    )
                    nc.vector.tensor_tensor(
                        out=msg[:, :nb, 0:D], in0=gjx,
                        in1=msg[:, :nb, D : D + 1].to_broadcast([P, nb, D]),
                        op=mybir.AluOpType.mult,
                    )
                    # scatter sub-chunks: unique rows within each; rotate accums
                    for s in range(b // SB):
                        ai = sc_counter % cfg.n_accums
                        acc = accums[ai]
                        sc_counter += 1
                        msub = msg[:, s * (SB // P) : (s + 1) * (SB // P), :]
                        rsub = rt[:, s * (SB // 16) : (s + 1) * (SB // 16)]
                        nc.gpsimd.dma_scatter_add(
                            acc[:, 0 : D + 1], msub, rsub, SB, SB, D + 1,
                            elem_step=DD, queue_num=ai % cfg.swdge_queues,
                        )
                    c0 += b

            # ---- final phase: out = (num/(den+eps)) @ W^T + b, 4 tiles/iter ----
            out_v = out.rearrange("(t p) d -> p t d", p=P)
            acc_vs = [a.rearrange("(t p) d -> p t d", p=P) for a in accums]
            for t0 in range(0, acc_t, 4):
                nt = min(4, acc_t - t0)
                a = gpool.tile([P, 4, D + 1], F32, tag="zi")
                nc.sync.dma_start(
                    out=a[:, :nt, :], in_=acc_vs[0][:, t0 : t0 + nt, 0 : D + 1]
                )
                for s in range(1, cfg.n_accums):
                    a2 = gpool.tile([P, 4, D + 1], F32, tag="gj")
                    nc.sync.dma_start(
                        out=a2[:, :nt, :], in_=acc_vs[s][:, t0 : t0 + nt, 0 : D + 1]
                    )
                    nc.vector.tensor_tensor(
                        out=a[:, :nt, :], in0=a[:, :nt, :], in1=a2[:, :nt, :],
                        op=mybir.AluOpType.add,
                    )
                dplus = spool.tile([P, 4], F32, tag="dplus")
                nc.vector.tensor_scalar_add(
                    dplus[:, :nt], a[:, :nt, D : D + 1], EPS_DENOM
                )
                rr = spool.tile([P, 4], F32, tag="rr")
                nc.vector.reciprocal(out=rr[:, :nt], in_=dplus[:, :nt])
                m = wpool.tile([P, 4, D + 1], F32, tag="prod")
                nc.vector.tensor_tensor(
                    out=m[:, :nt, 0:D], in0=a[:, :nt, 0:D],
                    in1=rr[:, :nt].to_broadcast([P, nt, D]),
                    op=mybir.AluOpType.mult,
                )
                nc.vector.memset(m[:, :nt, D : D + 1], 1.0)
                o = wpool.tile([P, 4, D], F32, tag="msg")
                for i in range(nt):
                    tp = ppool.tile([D + 1, P], F32, tag="tp", space="PSUM")
                    nc.tensor.transpose(
                        out=tp[:], in_=m[:, i, :], identity=ident[:]
                    )
                    lhs = fpool.tile([D + 1, P], F32, tag="lhs")
                    nc.vector.tensor_copy(out=lhs[:], in_=tp[:])
                    y = ppool.tile([P, D], F32, tag="y", space="PSUM")
                    nc.tensor.matmul(
                        out=y[:], lhsT=lhs[:], rhs=wbs[:], start=True, stop=True
                    )
                    nc.scalar.copy(out=o[:, i, :], in_=y[:])
                nc.sync.dma_start(out=out_v[:, t0 : t0 + nt, :], in_=o[:, :nt, :])

    nc.compile()
    return nc


def _wrap16(a: np.ndarray) -> np.ndarray:
    # token i -> partition i%16, col i//16; replicated 8x to 128 partitions
    w = a.reshape(-1, 16).T
    return np.ascontiguousarray(np.tile(w, (8, 1)))


# ---------------------------------------------------------------------------
# v2: matmul edge-softmax pipeline (no scatter, 1 gather descriptor per edge)
# ---------------------------------------------------------------------------
F16 = mybir.dt.float16
CHUNK = 768  # tokens per gather chunk (6 sub-blocks; Cp spans 2 PSUM banks)
NSBC = CHUNK // 128


def _v2_chunks(caps):
    """Static chunk schedule: (tile, group, sub-block offset, chunk tokens)."""
    out = []
    sb = 0
    for t in range(len(caps)):
        for g in range(len(caps[t])):
            cap = caps[t][g]
            while cap > 0:
                nck = min(CHUNK, cap)
                out.append((t, g, sb, nck))
                sb += nck // P
                cap -= nck
    return out


def shard_inputs2(cfg: Cfg, x, z, edge_index):
    """Per-core data for the v2 pipeline.

    Edges sorted by (dest row-tile, col-group); per-(tile,group) cell padded
    to a multiple of 128 tokens with a STATIC capacity = max over cores (the
    SPMD program is shared). Pad tokens: col idx 0, lrow 300 (mask -> 0).
    """
    D = cfg.d
    NT = cfg.acc_rows // P  # row tiles per core (98)
    row = np.asarray(edge_index[0], dtype=np.int64)
    col = np.asarray(edge_index[1], dtype=np.int64)
    core = row // cfg.nodes_per_core
    lrow_full = row % cfg.nodes_per_core
    t = lrow_full // P
    lr = lrow_full % P
    g = col // cfg.col_group_size
    c = (col % cfg.col_group_size).astype(np.int16)

    ncell = NT * cfg.col_groups
    cell = t * cfg.col_groups + g
    key = core * ncell + cell
    counts = np.bincount(key, minlength=cfg.n_cores * ncell).reshape(
        cfg.n_cores, ncell
    )
    caps_flat = ((counts.max(axis=0) + P - 1) // P) * P  # [ncell]
    caps = caps_flat.reshape(NT, cfg.col_groups)
    # a tile with zero edges everywhere still needs one pad sub-block so its
    # PSUM accumulator gets written (out rows become b, matching reference)
    for tt in range(NT):
        if caps[tt].sum() == 0:
            caps[tt][0] = P
    caps_flat = caps.reshape(-1)
    starts = np.concatenate([[0], np.cumsum(caps_flat)[:-1]])  # token offsets
    TOK = int(caps_flat.sum())

    # slot of each edge: starts[cell] + rank within (core, cell)
    order = np.argsort(key, kind="stable")
    ks = key[order]
    new = np.r_[True, ks[1:] != ks[:-1]]
    gid = np.cumsum(new) - 1
    pos = np.arange(len(order))
    rank = pos - pos[new][gid]
    slot = starts[cell[order]] + rank

    cidx_all = np.zeros((cfg.n_cores, TOK), np.int16)
    lrow_all = np.full((cfg.n_cores, TOK), 300.0, np.float16)
    flat = core[order] * TOK + slot
    cidx_all.reshape(-1)[flat] = c[order]
    lrow_all.reshape(-1)[flat] = lr[order].astype(np.float16)

    # tables
    zf = np.asarray(z, np.float32)
    zn = zf / np.maximum(np.linalg.norm(zf, axis=1, keepdims=True), EPS_NORM)
    xf = np.asarray(x, np.float32)
    zxt = np.concatenate([zn, xf], axis=1).astype(np.float16)  # [N, 128]
    zxt_groups = [
        np.ascontiguousarray(
            zxt[gg * cfg.col_group_size : (gg + 1) * cfg.col_group_size]
        )
        for gg in range(cfg.col_groups)
    ]
    znp = np.zeros((cfg.n_cores * cfg.nodes_per_core + cfg.acc_rows, D), np.float32)
    znp[: cfg.n_nodes] = zn

    in_maps = []
    for k in range(cfg.n_cores):
        zhT = np.ascontiguousarray(
            znp[k * cfg.nodes_per_core : k * cfg.nodes_per_core + cfg.acc_rows].T
        ).astype(np.float16)  # [64, acc_rows]
        lrow_tab = np.ascontiguousarray(
            lrow_all[k].reshape(TOK // P, P).T
        )  # [128, NSB]
        in_maps.append(
            {
                "zhT": zhT,
                "cidx": _wrap16(cidx_all[k]),
                "lrow": lrow_tab,
                **{f"zxt{gg}": zxt_groups[gg] for gg in range(cfg.col_groups)},
            }
        )
    return in_maps, caps.tolist(), TOK


def build_program2(cfg: Cfg, alpha: float, caps, TOK: int, repeat: int = 1):
    """v2 SPMD program. Inputs per core:
    zxt{g} [col_group_size, 128] f16  ([z-hat | x] table, replicated)
    zhT    [64, acc_rows] f16         (transposed normalized row slab)
    cidx   [128, TOK//16] i16         (col idx per token, wrap16)
    lrow   [128, TOK//128] f16        (local row in tile per token; 300=pad)
    wb     [65, 64] f32               ([W^T; b])
    Output: out [acc_rows, 64] f32.
    """
    D = cfg.d
    NT = cfg.acc_rows // P
    NSB = TOK // P
    chunks = _v2_chunks(caps)

    nc = bacc.Bacc(
        "TRN2", target_bir_lowering=False, debug=False,
        num_swdge_queues=cfg.swdge_queues, dynamic_dma_scratch_size=cfg.dma_scratch,
    )

    zxg = [
        nc.dram_tensor(f"zxt{g}", [cfg.col_group_size, 2 * D], F16,
                       kind="ExternalInput").ap()
        for g in range(cfg.col_groups)
    ]
    zhT = nc.dram_tensor("zhT", [D, cfg.acc_rows], F16, kind="ExternalInput").ap()
    cidx = nc.dram_tensor("cidx", [P, TOK // 16], I16, kind="ExternalInput").ap()
    lrow = nc.dram_tensor("lrow", [P, NSB], F16, kind="ExternalInput").ap()
    wb = nc.dram_tensor("wb", [D + 1, D], F32, kind="ExternalInput").ap()
    out = nc.dram_tensor("out", [cfg.acc_rows, D], F32, kind="ExternalOutput").ap()

    with tile.TileContext(nc) as tc:
        with (
            tc.tile_pool(name="const", bufs=1) as cpool,
            tc.tile_pool(name="gath", bufs=cfg.gbufs) as gpool,
            tc.tile_pool(name="work", bufs=cfg.gbufs) as wpool,
            tc.tile_pool(name="zh", bufs=2) as zpool,
            tc.tile_pool(name="fin", bufs=2) as fpool,
            tc.tile_pool(name="pc", bufs=2, space="PSUM") as pcpool,
            tc.tile_pool(name="pa", bufs=1, space="PSUM") as papool,
            tc.tile_pool(name="px", bufs=1, space="PSUM") as pxpool,
            tc.tile_pool(name="pf", bufs=1, space="PSUM") as pfpool,
        ):
            nc.gpsimd.load_library(MLP_LIB)
            identh = cpool.tile([P, P], F16, tag="identh")
            make_identity(nc, identh[:])
            identf = cpool.tile([P, P], F32, tag="identf")
            make_identity(nc, identf[:])
            iota32 = cpool.tile([P, NSBC, P], mybir.dt.int32, tag="iota32")
            nc.gpsimd.iota(
                iota32[:], pattern=[[0, NSBC], [1, P]], base=0, channel_multiplier=0
            )
            iotaf = cpool.tile([P, NSBC, P], F16, tag="iotaf")
            nc.vector.tensor_copy(out=iotaf[:], in_=iota32[:])
            cb = cpool.tile([P, 1], F32, tag="cb")
            nc.vector.memset(cb[:], -abs(float(alpha)))
            wbs = cpool.tile([D + 1, D], F32, tag="wbs")
            nc.sync.dma_start(out=wbs[:], in_=wb[:, :])
            lrt = cpool.tile([P, NSB], F16, tag="lrt")
            nc.sync.dma_start(out=lrt[:], in_=lrow[:, :])
            cxt = cpool.tile([P, TOK // 16], I16, tag="cxt")
            nc.sync.dma_start(out=cxt[:], in_=cidx[:, :])

            for rep in range(repeat):
                ci = 0
                fin = []  # (tile, af-slot) pending finalize
                af4 = None
                ti = 0
                while ti < len(chunks):
                    t = chunks[ti][0]
                    tj = ti
                    while tj < len(chunks) and chunks[tj][0] == t:
                        tj += 1
                    tile_chunks = chunks[ti:tj]

                    if t % 8 == 0:
                        zh8 = zpool.tile([D, 8, P], F16, tag="zh8")
                        hi = min(8, NT - t)
                        nc.sync.dma_start(
                            out=zh8[:, :hi, :],
                            in_=zhT[:, t * P : (t + hi) * P].rearrange(
                                "d (a p) -> d a p", p=P
                            ),
                        )
                    zh = zh8[:, t % 8, :]
                    A = papool.tile([P, D + 1], F32, tag="A", space="PSUM")

                    n_sb_tile = sum(ck[3] // P for ck in tile_chunks)
                    sbi = 0
                    for (_, g, sb0, nck) in tile_chunks:
                        nsb = nck // P
                        off = sb0 * P
                        gt = gpool.tile([P, 1, CHUNK], F16, tag="gt")
                        nc.gpsimd.dma_gather(
                            gt[:, :, :nck], zxg[g][:, :],
                            cxt[:, off // 16 : (off + nck) // 16],
                            nck, nck, 2 * D, transpose=True,
                            queue_num=ci % cfg.swdge_queues,
                        )
                        ci += 1
                        Cp = pcpool.tile([P, NSBC, P], F32, tag="Cp", space="PSUM")
                        for s in range(nsb):
                            nc.tensor.matmul(
                                out=Cp[:, s, :],
                                lhsT=gt[0:D, 0, s * P : (s + 1) * P],
                                rhs=zh, start=True, stop=True,
                            )
                        E = wpool.tile([P, NSBC, P], F16, tag="E")
                        nc.scalar.activation(
                            out=E[:, :nsb, :], in_=Cp[:, :nsb, :],
                            func=mybir.ActivationFunctionType.Exp,
                            bias=cb[:], scale=float(alpha),
                        )
                        W = wpool.tile([P, NSBC, P], F16, tag="W")
                        nc.vector.tensor_tensor(
                            out=W[:, :nsb, :], in0=iotaf[:, :nsb, :],
                            in1=lrt[:, sb0 : sb0 + nsb].to_broadcast([P, nsb, P]),
                            op=mybir.AluOpType.is_equal,
                        )
                        nc.vector.tensor_tensor(
                            out=W[:, :nsb, :], in0=W[:, :nsb, :], in1=E[:, :nsb, :],
                            op=mybir.AluOpType.mult,
                        )
                        xt = pxpool.tile([P, NSBC, D], F16, tag="xt", space="PSUM")
                        for s in range(nsb):
                            nc.tensor.transpose(
                                out=xt[:, s, :],
                                in_=gt[D : 2 * D, 0, s * P : (s + 1) * P],
                                identity=identh[D : 2 * D, D : 2 * D],
                            )
                        xc1 = wpool.tile([P, NSBC, D + 1], F16, tag="xc1")
                        if cfg.xc1_on_dve:
                            nc.vector.tensor_copy(
                                out=xc1[:, :nsb, 0:D], in_=xt[:, :nsb, :]
                            )
                        else:
                            nc.scalar.copy(out=xc1[:, :nsb, 0:D], in_=xt[:, :nsb, :])
                        nc.vector.memset(xc1[:, :nsb, D : D + 1], 1.0)
                        for s in range(nsb):
                            nc.tensor.matmul(
                                out=A[:], lhsT=W[:, s, :], rhs=xc1[:, s, :],
                                start=(sbi == 0), stop=(sbi == n_sb_tile - 1),
                            )
                            sbi += 1

                    # stash tile accumulator; finalize in batches of 4
                    if af4 is None:
                        af4 = fpool.tile([P, 4, D + 1], F32, tag="af4")
                    slot = len(fin)
                    nc.vector.tensor_copy(out=af4[:, slot, :], in_=A[:])
                    fin.append(t)
                    ti = tj

                    if len(fin) == 4 or ti >= len(chunks):
                        nt = len(fin)
                        dplus = fpool.tile([P, 4], F32, tag="dplus")
                        nc.vector.tensor_scalar_add(
                            dplus[:, :nt], af4[:, :nt, D : D + 1], EPS_DENOM
                        )
                        rr = fpool.tile([P, 4], F32, tag="rr")
                        nc.vector.reciprocal(out=rr[:, :nt], in_=dplus[:, :nt])
                        m = fpool.tile([P, 4, D + 1], F32, tag="m")
                        nc.vector.tensor_tensor(
                            out=m[:, :nt, 0:D], in0=af4[:, :nt, 0:D],
                            in1=rr[:, :nt].to_broadcast([P, nt, D]),
                            op=mybir.AluOpType.mult,
                        )
                        nc.vector.memset(m[:, :nt, D : D + 1], 1.0)
                        o = fpool.tile([P, 4, D], F32, tag="o")
                        for i, tt in enumerate(fin):
                            tp = pfpool.tile([D + 1, P], F32, tag="tp", space="PSUM")
                            nc.tensor.transpose(
                                out=tp[:], in_=m[:, i, :], identity=identf[:]
                            )
                            lhs = fpool.tile([D + 1, P], F32, tag="lhs")
                            nc.vector.tensor_copy(out=lhs[:], in_=tp[:])
                            y = pfpool.tile([P, D], F32, tag="y", space="PSUM")
                            nc.tensor.matmul(
                                out=y[:], lhsT=lhs[:], rhs=wbs[:],
                                start=True, stop=True,
                            )
                            nc.scalar.copy(out=o[:, i, :], in_=y[:])
                        t0 = fin[0]
                        nc.sync.dma_start(
                            out=out[t0 * P : (t0 + nt) * P, :].rearrange(
                                "(a p) d -> p a d", p=P
                            ),
                            in_=o[:, :nt, :],
                        )
                        fin = []
                        af4 = None

    nc.compile()
    return nc


def run2(cfg: Cfg, x, edge_index, z, W, b, alpha, bias_edge, repeat=1):
    from concourse.bass_utils import run_bass_kernel_spmd

    in_maps, caps, TOK = shard_inputs2(cfg, x, z, edge_index)
    wb = np.ascontiguousarray(
        np.concatenate(
            [np.asarray(W, np.float32).T, np.asarray(b, np.float32)[None, :]], axis=0
        )
    )
    for m in in_maps:
        m["wb"] = wb
    nc = build_program2(cfg, float(np.asarray(alpha)), caps, TOK, repeat=repeat)
    core_ids = list(range(cfg.n_cores))
    res = run_bass_kernel_spmd(nc, in_maps, core_ids)
    out = np.concatenate(
        [res.results[k]["out"][: cfg.nodes_per_core] for k in core_ids], axis=0
    )[: cfg.n_nodes]
    return out.astype(np.float32), res


def shard_inputs(cfg: Cfg, x, z, edge_index):
    """Bucket edges by (owner core, col group); deal each row's edges across
    scatter sub-chunks so each scatter instruction has unique rows."""
    D = cfg.d
    row = np.asarray(edge_index[0], dtype=np.int64)
    col = np.asarray(edge_index[1], dtype=np.int64)
    E = row.shape[0]
    core = row // cfg.nodes_per_core
    grp = col // cfg.col_group_size
    bucket = core * cfg.col_groups + grp
    n_bins = cfg.n_cores * cfg.col_groups

    # rank of each edge within its (bucket, row) group
    o = np.lexsort((row, bucket))
    bs, rs = bucket[o], row[o]
    new = np.r_[True, (bs[1:] != bs[:-1]) | (rs[1:] != rs[:-1])]
    gid = np.cumsum(new) - 1
    pos = np.arange(E)
    firstpos = pos[new]
    rank = pos - firstpos[gid]
    maxmult = int(rank.max()) + 1 if E else 1

    # per-(bucket,row) group sizes and within-bucket exclusive cumsum: row r's
    # edges go to chunks (start_r + rank) % n — balanced to +-1 per bucket and
    # unique within each chunk (mult <= n_chunks)
    mult = np.bincount(gid)
    g_bucket = bs[new]
    g_cum = np.concatenate([[0], np.cumsum(mult)[:-1]])
    g_new_bucket = np.r_[True, g_bucket[1:] != g_bucket[:-1]]
    bucket_base = np.maximum.accumulate(np.where(g_new_bucket, g_cum, 0))
    g_start = g_cum - bucket_base
    bucket_counts = np.bincount(bs, minlength=n_bins)
    maxcount = int(bucket_counts.max()) if E else 1

    n_chunks = max(
        (maxcount + cfg.scatter_b - 1) // cfg.scatter_b, maxmult, 1
    )
    while True:
        chunkid = (g_start[gid] + rank) % n_chunks
        cc = np.bincount(bs * n_chunks + chunkid, minlength=n_bins * n_chunks)
        if maxmult <= n_chunks and cc.max() <= cfg.scatter_b:
            break
        n_chunks += 1
    eff = dataclasses.replace(cfg, tokens_per_group=n_chunks * cfg.scatter_b)

    # slot within (bucket, chunk)
    o2 = np.lexsort((chunkid, bs))
    b2, c2 = bs[o2], chunkid[o2]
    new2 = np.r_[True, (b2[1:] != b2[:-1]) | (c2[1:] != c2[:-1])]
    gid2 = np.cumsum(new2) - 1
    firstpos2 = pos[new2]
    rank2 = pos - firstpos2[gid2]
    tokpos = c2 * cfg.scatter_b + rank2

    TG = eff.tokens_per_group
    rl_all = np.full((n_bins, TG), eff.junk_row, np.int16)
    cl_all = np.zeros((n_bins, TG), np.int16)
    edge_sorted = o[o2]  # original edge ids in placement order
    flat = b2 * TG + tokpos
    rl_all.reshape(-1)[flat] = (row[edge_sorted] % cfg.nodes_per_core).astype(np.int16)
    cl_all.reshape(-1)[flat] = (col[edge_sorted] % cfg.col_group_size).astype(np.int16)

    zx = np.concatenate(
        [np.asarray(z, np.float32), np.asarray(x, np.float32)], axis=1
    )
    zx_groups = [
        np.ascontiguousarray(zx[g * cfg.col_group_size : (g + 1) * cfg.col_group_size])
        for g in range(cfg.col_groups)
    ]
    zpad = np.zeros((cfg.n_cores * cfg.nodes_per_core + cfg.acc_rows, D), np.float32)
    zpad[: cfg.n_nodes] = np.asarray(z, np.float32)

    in_maps = []
    for k in range(cfg.n_cores):
        ridx_g = np.stack(
            [_wrap16(rl_all[k * cfg.col_groups + g]) for g in range(cfg.col_groups)]
        )
        cidx_g = np.stack(
            [_wrap16(cl_all[k * cfg.col_groups + g]) for g in range(cfg.col_groups)]
        )
        in_maps.append(
            {
                "zrow": np.ascontiguousarray(
                    zpad[k * cfg.nodes_per_core : k * cfg.nodes_per_core + cfg.acc_rows]
                ),
                **{f"zx{g}": zx_groups[g] for g in range(cfg.col_groups)},
                "ridx": ridx_g,
                "cidx": cidx_g,
            }
        )
    return in_maps, eff


def run(cfg: Cfg, x, edge_index, z, W, b, alpha, bias_edge, trace=False):
    from concourse.bass_utils import run_bass_kernel_spmd

    in_maps, eff = shard_inputs(cfg, x, z, edge_index)
    wb = np.ascontiguousarray(
        np.concatenate(
            [np.asarray(W, np.float32).T, np.asarray(b, np.float32)[None, :]], axis=0
        )
    )
    for m in in_maps:
        m["wb"] = wb
    nc = build_program(eff, float(np.asarray(alpha)))
    core_ids = list(range(eff.n_cores))
    res = run_bass_kernel_spmd(nc, in_maps, core_ids, trace=trace)
    out = np.concatenate(
        [res.results[k]["out"][: eff.nodes_per_core] for k in core_ids], axis=0
    )[: eff.n_nodes]
    return out.astype(np.float32), res


def kernel(**inputs) -> np.ndarray:
    out, _ = run2(
        FULL,
        inputs["x"],
        inputs["edge_index"],
        inputs["z"],
        inputs["W"],
        inputs["b"],
        inputs["alpha"],
        inputs["bias_edge"],
    )
    return out



# revision 21
# speedup vs baseline: 1.3557x; 1.3557x over previous
"""Trainium2 Bass kernel for nn_DiracGraphConv (GNN edge-softmax message passing).

Strategy (8 NeuronCores, SPMD, no collectives):
  - Shard edges by DESTINATION node range: core k owns nodes
    [k*12500, (k+1)*12500) and processes exactly the edges whose row
    (destination) falls in that range. Segment-sums for a node happen
    entirely on its owner core, so per-core results are disjoint node
    slabs and the full output is a host-side concatenation.
  - Within a core, edges are bucketed by col//25000 into 4 groups so
    gather indices fit int16 (dma_gather/dma_scatter_add contract).
  - The core's z slab is L2-normalized once on device (zh table), so the
    per-edge cosine needs only num = zh[row]&middot;z[col] and |z[col]|:
    corr = num / max(|z_col|, eps). exp shift constant is |alpha|
    (softmax shift invariance; bias_edge cancels).
  - Per gather-chunk: dma_gather zh[row] (row-local slab) and zx[col]
    (combined [z | x] 512-byte rows), compute logits and exp on DVE/ACT
    (exp lands directly in the message's 65th column), then
    dma_scatter_add the 65-wide message [e * x[col], e] into a per-core
    DRAM accumulator.
  - HW dma_scatter_add races on duplicate indices within an instruction
    (and across concurrently-flying instructions) — verified on HW.
    Countermeasures:
    (a) the host deals each (core,group,row)'s edges round-robin across
        scatter sub-chunks, so every scatter instruction carries unique
        row indices (pad tokens all hit a junk row; races there are
        harmless);
    (b) scatter instructions rotate across n_accums accumulator buffers;
        scatters on the same buffer are WAW-serialized by Tile sems, so
        same-row descriptors from different instructions are never in
        flight together. Final phase sums the buffers.
  - Final phase (batched 4 node-tiles per iteration):
    out = (num / (denom + eps)) @ W^T + b via PE transpose + matmul with
    [W^T; b] and an appended ones-column.
"""

import sys

sys.path.insert(0, "/opt/trn_rl_repo")

import dataclasses
from dataclasses import dataclass

import numpy as np

from concourse import bacc, bass, mybir, tile
from concourse.library_config import mlp as MLP_LIB
from concourse.masks import make_identity

P = 128
F32 = mybir.dt.float32
I16 = mybir.dt.int16
EPS_DENOM = 1e-9
EPS_NORM = 1e-9


@dataclass(frozen=True)
class Cfg:
    n_cores: int = 8
    n_nodes: int = 100000
    d: int = 64
    nodes_per_core: int = 12500
    col_groups: int = 4
    col_group_size: int = 25000
    # SWDGE carveout fits scratch/16 descriptors per instruction (16B/desc) —
    # every dma_gather/dma_scatter_add must stay below that.
    tokens_per_group: int = 52224  # multiple of scatter_b (auto-grown if needed)
    gather_b: int = 768  # max tokens per gather/compute chunk (<1024 descs)
    scatter_b: int = 768  # tokens per scatter instruction (unique rows)
    n_accums: int = 4
    swdge_queues: int = 2
    dma_scratch: int = 16384
    gbufs: int = 6
    xc1_on_dve: bool = False

    @property
    def acc_rows(self) -> int:
        # accumulator rows: nodes_per_core real + 1 junk row, padded to 128
        return ((self.nodes_per_core + 1 + P - 1) // P) * P

    @property
    def junk_row(self) -> int:
        return self.nodes_per_core

    @property
    def n_scatter_chunks(self) -> int:
        return self.tokens_per_group // self.scatter_b

    def gather_chunks(self):
        sizes = []
        t = self.tokens_per_group
        while t > 0:
            b = min(self.gather_b, t)
            sizes.append(b)
            t -= b
        assert all(s % self.scatter_b == 0 for s in sizes)
        return sizes


FULL = Cfg()


def build_program(cfg: Cfg, alpha: float, repeat: int = 1):
    """One SPMD program for all cores. Inputs (per core):
    zrow [acc_rows, d] f32   core's raw z slab
    zx   [col_groups*col_group_size, 2d] f32   full [z | x] table
    ridx [col_groups, 128, tokens_per_group//16] i16
    cidx [col_groups, 128, tokens_per_group//16] i16
    wb   [d+1, d] f32  ([W^T; b])
    Output: out [acc_rows, d] f32 (rows >= nodes_per_core are garbage)
    """
    D = cfg.d
    DD = 2 * D
    TG16 = cfg.tokens_per_group // 16
    SB = cfg.scatter_b

    nc = bacc.Bacc(
        "TRN2", target_bir_lowering=False, debug=False,
        num_swdge_queues=cfg.swdge_queues, dynamic_dma_scratch_size=cfg.dma_scratch,
    )

    zrow = nc.dram_tensor("zrow", [cfg.acc_rows, D], F32, kind="ExternalInput").ap()
    zxg = [
        nc.dram_tensor(f"zx{g}", [cfg.col_group_size, DD], F32, kind="ExternalInput").ap()
        for g in range(cfg.col_groups)
    ]
    ridx = nc.dram_tensor(
        "ridx", [cfg.col_groups, P, TG16], I16, kind="ExternalInput"
    ).ap()
    cidx = nc.dram_tensor(
        "cidx", [cfg.col_groups, P, TG16], I16, kind="ExternalInput"
    ).ap()
    wb = nc.dram_tensor("wb", [D + 1, D], F32, kind="ExternalInput").ap()
    out = nc.dram_tensor("out", [cfg.acc_rows, D], F32, kind="ExternalOutput").ap()

    zh = nc.dram_tensor("zh", [cfg.acc_rows, D], F32).ap()
    accums = [
        nc.dram_tensor(f"accum{s}", [cfg.acc_rows, DD], F32).ap()
        for s in range(cfg.n_accums)
    ]

    with tile.TileContext(nc) as tc:
        with (
            tc.tile_pool(name="const", bufs=1) as cpool,
            tc.tile_pool(name="idx", bufs=3) as ipool,
            tc.tile_pool(name="gath", bufs=2) as gpool,
            tc.tile_pool(name="work", bufs=2) as wpool,
            tc.tile_pool(name="smal", bufs=3) as spool,
            tc.tile_pool(name="fin", bufs=2) as fpool,
            tc.tile_pool(name="psum", bufs=2, space="PSUM") as ppool,
        ):
            # ---- constants ----
            nc.gpsimd.load_library(MLP_LIB)
            cb = cpool.tile([P, 1], F32, tag="cb")
            nc.vector.memset(cb[:], -abs(float(alpha)))
            ident = cpool.tile([P, P], F32, tag="ident")
            make_identity(nc, ident[:])
            wbs = cpool.tile([D + 1, D], F32, tag="wbs")
            nc.sync.dma_start(out=wbs[:], in_=wb[:, :])

            # ---- normalize the row slab: zh = zrow / max(|zrow|, eps) ----
            r0 = 0
            while r0 < cfg.acc_rows:
                j = min(8, (cfg.acc_rows - r0) // P)
                rows = slice(r0, r0 + j * P)
                zt_in = gpool.tile([P, 8, D], F32, tag="zi")
                nc.sync.dma_start(
                    out=zt_in[:, :j, :],
                    in_=zrow[rows, :].rearrange("(p a) d -> p a d", p=P),
                )
                sq = wpool.tile([P, 8, D], F32, tag="prod")
                nc.vector.tensor_tensor(
                    out=sq[:, :j, :], in0=zt_in[:, :j, :], in1=zt_in[:, :j, :],
                    op=mybir.AluOpType.mult,
                )
                ns = spool.tile([P, 8], F32, tag="ns")
                nc.vector.tensor_reduce(
                    out=ns[:, :j], in_=sq[:, :j, :], axis=mybir.AxisListType.X,
                    op=mybir.AluOpType.add,
                )
                nc.vector.tensor_scalar_max(ns[:, :j], ns[:, :j], 1e-18)
                nc.scalar.sqrt(out=ns[:, :j], in_=ns[:, :j])
                rr = spool.tile([P, 8], F32, tag="nr")
                nc.vector.reciprocal(out=rr[:, :j], in_=ns[:, :j])
                zo = gpool.tile([P, 8, D], F32, tag="gj")
                nc.vector.tensor_tensor(
                    out=zo[:, :j, :], in0=zt_in[:, :j, :],
                    in1=rr[:, :j].to_broadcast([P, j, D]), op=mybir.AluOpType.mult,
                )
                nc.sync.dma_start(
                    out=zh[rows, :].rearrange("(p a) d -> p a d", p=P),
                    in_=zo[:, :j, :],
                )
                r0 += j * P

            # ---- zero the accumulators ----
            acc_t = cfg.acc_rows // P
            zt = cpool.tile([P, 8 * DD], F32, tag="zt")
            nc.vector.memset(zt[:], 0.0)
            for acc in accums:
                acc_v = acc.rearrange("(t p) d -> p t d", p=P)
                for t0 in range(0, acc_t, 8):
                    nt = min(8, acc_t - t0)
                    nc.sync.dma_start(
                        out=acc_v[:, t0 : t0 + nt, :],
                        in_=zt[:, : nt * DD].rearrange("p (t d) -> p t d", d=DD),
                    )

            # ---- edge phase ----
            sc_counter = 0
            ci = 0
            for g in range(cfg.col_groups * repeat):
                g = g % cfg.col_groups
                zx_win = zxg[g][:, :]
                c0 = 0
                for b in cfg.gather_chunks():
                    nb = b // P
                    q = ci % cfg.swdge_queues
                    ci += 1
                    s16 = slice(c0 // 16, (c0 + b) // 16)
                    rt = ipool.tile([P, cfg.gather_b // 16], I16, tag="rt")
                    ct = ipool.tile([P, cfg.gather_b // 16], I16, tag="ct")
                    nc.sync.dma_start(out=rt[:, : b // 16], in_=ridx[g, :, s16])
                    nc.sync.dma_start(out=ct[:, : b // 16], in_=cidx[g, :, s16])

                    zi = gpool.tile([P, cfg.gather_b // P, D], F32, tag="zi")
                    gj = gpool.tile([P, cfg.gather_b // P, DD], F32, tag="gj")
                    nc.gpsimd.dma_gather(
                        zi[:, :nb, :], zh[:, :], rt[:, : b // 16], b, b, D,
                        queue_num=q,
                    )
                    nc.gpsimd.dma_gather(
                        gj[:, :nb, :], zx_win, ct[:, : b // 16], b, b, DD,
                        queue_num=q,
                    )
                    gjz = gj[:, :nb, 0:D]
                    gjx = gj[:, :nb, D:DD]

                    prod = wpool.tile([P, cfg.gather_b // P, D], F32, tag="prod")
                    num = spool.tile([P, cfg.gather_b // P], F32, tag="num")
                    nj = spool.tile([P, cfg.gather_b // P], F32, tag="nj")
                    nc.vector.tensor_tensor(
                        out=prod[:, :nb, :], in0=zi[:, :nb, :], in1=gjz,
                        op=mybir.AluOpType.mult,
                    )
                    nc.vector.tensor_reduce(
                        out=num[:, :nb], in_=prod[:, :nb, :],
                        axis=mybir.AxisListType.X, op=mybir.AluOpType.add,
                    )
                    nc.vector.tensor_tensor(
                        out=prod[:, :nb, :], in0=gjz, in1=gjz,
                        op=mybir.AluOpType.mult,
                    )
                    nc.vector.tensor_reduce(
                        out=nj[:, :nb], in_=prod[:, :nb, :],
                        axis=mybir.AxisListType.X, op=mybir.AluOpType.add,
                    )
                    nc.vector.tensor_scalar_max(nj[:, :nb], nj[:, :nb], 1e-18)
                    nc.scalar.sqrt(out=nj[:, :nb], in_=nj[:, :nb])
                    rcp = spool.tile([P, cfg.gather_b // P], F32, tag="rcp")
                    nc.vector.reciprocal(out=rcp[:, :nb], in_=nj[:, :nb])
                    corr = spool.tile([P, cfg.gather_b // P], F32, tag="corr")
                    nc.vector.tensor_tensor(
                        out=corr[:, :nb], in0=num[:, :nb], in1=rcp[:, :nb],
                        op=mybir.AluOpType.mult,
                    )
                    msg = wpool.tile([P, cfg.gather_b // P, D + 1], F32, tag="msg")
                    # exp lands directly in the message's denom column
                    nc.scalar.activation(
                        out=msg[:, :nb, D : D + 1], in_=corr[:, :nb],
                        func=mybir.ActivationFunctionType.Exp,
                        bias=cb[:], scale=float(alpha),
                    )
                    nc.vector.tensor_tensor(
                        out=msg[:, :nb, 0:D], in0=gjx,
                        in1=msg[:, :nb, D : D + 1].to_broadcast([P, nb, D]),
                        op=mybir.AluOpType.mult,
                    )
                    # scatter sub-chunks: unique rows within each; rotate accums
                    for s in range(b // SB):
                        ai = sc_counter % cfg.n_accums
                        acc = accums[ai]
                        sc_counter += 1
                        msub = msg[:, s * (SB // P) : (s + 1) * (SB // P), :]
                        rsub = rt[:, s * (SB // 16) : (s + 1) * (SB // 16)]
                        nc.gpsimd.dma_scatter_add(
                            acc[:, 0 : D + 1], msub, rsub, SB, SB, D + 1,
                            elem_step=DD, queue_num=ai % cfg.swdge_queues,
                        )
                    c0 += b

            # ---- final phase: out = (num/(den+eps)) @ W^T + b, 4 tiles/iter ----
            out_v = out.rearrange("(t p) d -> p t d", p=P)
            acc_vs = [a.rearrange("(t p) d -> p t d", p=P) for a in accums]
            for t0 in range(0, acc_t, 4):
                nt = min(4, acc_t - t0)
                a = gpool.tile([P, 4, D + 1], F32, tag="zi")
                nc.sync.dma_start(
                    out=a[:, :nt, :], in_=acc_vs[0][:, t0 : t0 + nt, 0 : D + 1]
                )
                for s in range(1, cfg.n_accums):
                    a2 = gpool.tile([P, 4, D + 1], F32, tag="gj")
                    nc.sync.dma_start(
                        out=a2[:, :nt, :], in_=acc_vs[s][:, t0 : t0 + nt, 0 : D + 1]
                    )
                    nc.vector.tensor_tensor(
                        out=a[:, :nt, :], in0=a[:, :nt, :], in1=a2[:, :nt, :],
                        op=mybir.AluOpType.add,
                    )
                dplus = spool.tile([P, 4], F32, tag="dplus")
                nc.vector.tensor_scalar_add(
                    dplus[:, :nt], a[:, :nt, D : D + 1], EPS_DENOM
                )
                rr = spool.tile([P, 4], F32, tag="rr")
                nc.vector.reciprocal(out=rr[:, :nt], in_=dplus[:, :nt])
                m = wpool.tile([P, 4, D + 1], F32, tag="prod")
                nc.vector.tensor_tensor(
                    out=m[:, :nt, 0:D], in0=a[:, :nt, 0:D],
                    in1=rr[:, :nt].to_broadcast([P, nt, D]),
                    op=mybir.AluOpType.mult,
                )
                nc.vector.memset(m[:, :nt, D : D + 1], 1.0)
                o = wpool.tile([P, 4, D], F32, tag="msg")
                for i in range(nt):
                    tp = ppool.tile([D + 1, P], F32, tag="tp", space="PSUM")
                    nc.tensor.transpose(
                        out=tp[:], in_=m[:, i, :], identity=ident[:]
                    )
                    lhs = fpool.tile([D + 1, P], F32, tag="lhs")
                    nc.vector.tensor_copy(out=lhs[:], in_=tp[:])
                    y = ppool.tile([P, D], F32, tag="y", space="PSUM")
                    nc.tensor.matmul(
                        out=y[:], lhsT=lhs[:], rhs=wbs[:], start=True, stop=True
                    )
                    nc.scalar.copy(out=o[:, i, :], in_=y[:])
                nc.sync.dma_start(out=out_v[:, t0 : t0 + nt, :], in_=o[:, :nt, :])

    nc.compile()
    return nc


def _wrap16(a: np.ndarray) -> np.ndarray:
    # token i -> partition i%16, col i//16; replicated 8x to 128 partitions
    w = a.reshape(-1, 16).T
    return np.ascontiguousarray(np.tile(w, (8, 1)))


# ---------------------------------------------------------------------------
# v2: matmul edge-softmax pipeline (no scatter, 1 gather descriptor per edge)
# ---------------------------------------------------------------------------
F16 = mybir.dt.float16
CHUNK = 768  # tokens per gather chunk (6 sub-blocks; Cp spans 2 PSUM banks)
NSBC = CHUNK // 128


def _v2_chunks(caps):
    """Static chunk schedule: (tile, group, sub-block offset, chunk tokens)."""
    out = []
    sb = 0
    for t in range(len(caps)):
        for g in range(len(caps[t])):
            cap = caps[t][g]
            while cap > 0:
                nck = min(CHUNK, cap)
                out.append((t, g, sb, nck))
                sb += nck // P
                cap -= nck
    return out


def shard_inputs2(cfg: Cfg, x, z, edge_index):
    """Per-core data for the v2 pipeline.

    Edges sorted by (dest row-tile, col-group); per-(tile,group) cell padded
    to a multiple of 128 tokens with a STATIC capacity = max over cores (the
    SPMD program is shared). Pad tokens: col idx 0, lrow 300 (mask -> 0).
    """
    D = cfg.d
    NT = cfg.acc_rows // P  # row tiles per core (98)
    row = np.asarray(edge_index[0], dtype=np.int64)
    col = np.asarray(edge_index[1], dtype=np.int64)
    core = row // cfg.nodes_per_core
    lrow_full = row % cfg.nodes_per_core
    t = lrow_full // P
    lr = lrow_full % P
    g = col // cfg.col_group_size
    c = (col % cfg.col_group_size).astype(np.int16)

    ncell = NT * cfg.col_groups
    cell = t * cfg.col_groups + g
    key = core * ncell + cell
    counts = np.bincount(key, minlength=cfg.n_cores * ncell).reshape(
        cfg.n_cores, ncell
    )
    caps_flat = ((counts.max(axis=0) + P - 1) // P) * P  # [ncell]
    caps = caps_flat.reshape(NT, cfg.col_groups)
    # a tile with zero edges everywhere still needs one pad sub-block so its
    # PSUM accumulator gets written (out rows become b, matching reference)
    for tt in range(NT):
        if caps[tt].sum() == 0:
            caps[tt][0] = P
    caps_flat = caps.reshape(-1)
    starts = np.concatenate([[0], np.cumsum(caps_flat)[:-1]])  # token offsets
    TOK = int(caps_flat.sum())

    # slot of each edge: starts[cell] + rank within (core, cell)
    order = np.argsort(key, kind="stable")
    ks = key[order]
    new = np.r_[True, ks[1:] != ks[:-1]]
    gid = np.cumsum(new) - 1
    pos = np.arange(len(order))
    rank = pos - pos[new][gid]
    slot = starts[cell[order]] + rank

    cidx_all = np.zeros((cfg.n_cores, TOK), np.int16)
    lrow_all = np.full((cfg.n_cores, TOK), 300.0, np.float16)
    flat = core[order] * TOK + slot
    cidx_all.reshape(-1)[flat] = c[order]
    lrow_all.reshape(-1)[flat] = lr[order].astype(np.float16)

    # tables
    zf = np.asarray(z, np.float32)
    zn = zf / np.maximum(np.linalg.norm(zf, axis=1, keepdims=True), EPS_NORM)
    xf = np.asarray(x, np.float32)
    zxt = np.concatenate([zn, xf], axis=1).astype(np.float16)  # [N, 128]
    zxt_groups = [
        np.ascontiguousarray(
            zxt[gg * cfg.col_group_size : (gg + 1) * cfg.col_group_size]
        )
        for gg in range(cfg.col_groups)
    ]
    znp = np.zeros((cfg.n_cores * cfg.nodes_per_core + cfg.acc_rows, D), np.float32)
    znp[: cfg.n_nodes] = zn

    in_maps = []
    for k in range(cfg.n_cores):
        zhT = np.ascontiguousarray(
            znp[k * cfg.nodes_per_core : k * cfg.nodes_per_core + cfg.acc_rows].T
        ).astype(np.float16)  # [64, acc_rows]
        lrow_tab = np.ascontiguousarray(
            lrow_all[k].reshape(TOK // P, P).T
        )  # [128, NSB]
        in_maps.append(
            {
                "zhT": zhT,
                "cidx": _wrap16(cidx_all[k]),
                "lrow": lrow_tab,
                **{f"zxt{gg}": zxt_groups[gg] for gg in range(cfg.col_groups)},
            }
        )
    return in_maps, caps.tolist(), TOK


def build_program2(cfg: Cfg, alpha: float, caps, TOK: int, repeat: int = 1):
    """v2 SPMD program. Inputs per core:
    zxt{g} [col_group_size, 128] f16  ([z-hat | x] table, replicated)
    zhT    [64, acc_rows] f16         (transposed normalized row slab)
    cidx   [128, TOK//16] i16         (col idx per token, wrap16)
    lrow   [128, TOK//128] f16        (local row in tile per token; 300=pad)
    wb     [65, 64] f32               ([W^T; b])
    Output: out [acc_rows, 64] f32.
    """
    D = cfg.d
    NT = cfg.acc_rows // P
    NSB = TOK // P
    chunks = _v2_chunks(caps)

    nc = bacc.Bacc(
        "TRN2", target_bir_lowering=False, debug=False,
        num_swdge_queues=cfg.swdge_queues, dynamic_dma_scratch_size=cfg.dma_scratch,
    )
    if cfg.swdge_queues > 1:
        # Shield the SBUF region right after the 16KB SWDGE scratch: with
        # multiple queues the extra queues' descriptor rings were observed to
        # intermittently corrupt whatever data lives there (device crash).
        nc.alloc_sbuf_tensor(
            "qring_shield",
            [P, (cfg.swdge_queues - 1) * cfg.dma_scratch],
            mybir.dt.uint8,
        )

    zxg = [
        nc.dram_tensor(f"zxt{g}", [cfg.col_group_size, 2 * D], F16,
                       kind="ExternalInput").ap()
        for g in range(cfg.col_groups)
    ]
    zhT = nc.dram_tensor("zhT", [D, cfg.acc_rows], F16, kind="ExternalInput").ap()
    cidx = nc.dram_tensor("cidx", [P, TOK // 16], I16, kind="ExternalInput").ap()
    lrow = nc.dram_tensor("lrow", [P, NSB], F16, kind="ExternalInput").ap()
    wb = nc.dram_tensor("wb", [D + 1, D], F32, kind="ExternalInput").ap()
    out = nc.dram_tensor("out", [cfg.acc_rows, D], F32, kind="ExternalOutput").ap()

    with tile.TileContext(nc) as tc:
        with (
            tc.tile_pool(name="const", bufs=1) as cpool,
            tc.tile_pool(name="gath", bufs=cfg.gbufs) as gpool,
            tc.tile_pool(name="work", bufs=cfg.gbufs) as wpool,
            tc.tile_pool(name="zh", bufs=2) as zpool,
            tc.tile_pool(name="fin", bufs=2) as fpool,
            tc.tile_pool(name="pc", bufs=2, space="PSUM") as pcpool,
            tc.tile_pool(name="pa", bufs=1, space="PSUM") as papool,
            tc.tile_pool(name="px", bufs=1, space="PSUM") as pxpool,
            tc.tile_pool(name="pf", bufs=1, space="PSUM") as pfpool,
        ):
            nc.gpsimd.load_library(MLP_LIB)
            identh = cpool.tile([P, P], F16, tag="identh")
            make_identity(nc, identh[:])
            identf = cpool.tile([P, P], F32, tag="identf")
            make_identity(nc, identf[:])
            iota32 = cpool.tile([P, NSBC, P], mybir.dt.int32, tag="iota32")
            nc.gpsimd.iota(
                iota32[:], pattern=[[0, NSBC], [1, P]], base=0, channel_multiplier=0
            )
            iotaf = cpool.tile([P, NSBC, P], F16, tag="iotaf")
            nc.vector.tensor_copy(out=iotaf[:], in_=iota32[:])
            cb = cpool.tile([P, 1], F32, tag="cb")
            nc.vector.memset(cb[:], -abs(float(alpha)))
            wbs = cpool.tile([D + 1, D], F32, tag="wbs")
            nc.sync.dma_start(out=wbs[:], in_=wb[:, :])
            lrt = cpool.tile([P, NSB], F16, tag="lrt")
            nc.sync.dma_start(out=lrt[:], in_=lrow[:, :])
            cxt = cpool.tile([P, TOK // 16], I16, tag="cxt")
            nc.sync.dma_start(out=cxt[:], in_=cidx[:, :])

            for rep in range(repeat):
                ci = 0
                fin = []  # (tile, af-slot) pending finalize
                af4 = None
                ti = 0
                while ti < len(chunks):
                    t = chunks[ti][0]
                    tj = ti
                    while tj < len(chunks) and chunks[tj][0] == t:
                        tj += 1
                    tile_chunks = chunks[ti:tj]

                    if t % 8 == 0:
                        zh8 = zpool.tile([D, 8, P], F16, tag="zh8")
                        hi = min(8, NT - t)
                        nc.sync.dma_start(
                            out=zh8[:, :hi, :],
                            in_=zhT[:, t * P : (t + hi) * P].rearrange(
                                "d (a p) -> d a p", p=P
                            ),
                        )
                    zh = zh8[:, t % 8, :]
                    A = papool.tile([P, D + 1], F32, tag="A", space="PSUM")

                    n_sb_tile = sum(ck[3] // P for ck in tile_chunks)
                    sbi = 0
                    for (_, g, sb0, nck) in tile_chunks:
                        nsb = nck // P
                        off = sb0 * P
                        gt = gpool.tile([P, 1, CHUNK], F16, tag="gt")
                        nc.gpsimd.dma_gather(
                            gt[:, :, :nck], zxg[g][:, :],
                            cxt[:, off // 16 : (off + nck) // 16],
                            nck, nck, 2 * D, transpose=True,
                            queue_num=ci % cfg.swdge_queues,
                        )
                        ci += 1
                        Cp = pcpool.tile([P, NSBC, P], F32, tag="Cp", space="PSUM")
                        for s in range(nsb):
                            nc.tensor.matmul(
                                out=Cp[:, s, :],
                                lhsT=gt[0:D, 0, s * P : (s + 1) * P],
                                rhs=zh, start=True, stop=True,
                            )
                        E = wpool.tile([P, NSBC, P], F16, tag="E")
                        nc.scalar.activation(
                            out=E[:, :nsb, :], in_=Cp[:, :nsb, :],
                            func=mybir.ActivationFunctionType.Exp,
                            bias=cb[:], scale=float(alpha),
                        )
                        W = wpool.tile([P, NSBC, P], F16, tag="W")
                        nc.vector.tensor_tensor(
                            out=W[:, :nsb, :], in0=iotaf[:, :nsb, :],
                            in1=lrt[:, sb0 : sb0 + nsb].to_broadcast([P, nsb, P]),
                            op=mybir.AluOpType.is_equal,
                        )
                        nc.vector.tensor_tensor(
                            out=W[:, :nsb, :], in0=W[:, :nsb, :], in1=E[:, :nsb, :],
                            op=mybir.AluOpType.mult,
                        )
                        xt = pxpool.tile([P, NSBC, D], F16, tag="xt", space="PSUM")
                        for s in range(nsb):
                            nc.tensor.transpose(
                                out=xt[:, s, :],
                                in_=gt[D : 2 * D, 0, s * P : (s + 1) * P],
                                identity=identh[D : 2 * D, D : 2 * D],
                            )
                        xc1 = wpool.tile([P, NSBC, D + 1], F16, tag="xc1")
                        if cfg.xc1_on_dve:
                            nc.vector.tensor_copy(
                                out=xc1[:, :nsb, 0:D], in_=xt[:, :nsb, :]
                            )
                        else:
                            nc.scalar.copy(out=xc1[:, :nsb, 0:D], in_=xt[:, :nsb, :])
                        nc.vector.memset(xc1[:, :nsb, D : D + 1], 1.0)
                        for s in range(nsb):
                            nc.tensor.matmul(
                                out=A[:], lhsT=W[:, s, :], rhs=xc1[:, s, :],
                                start=(sbi == 0), stop=(sbi == n_sb_tile - 1),
                            )
                            sbi += 1

                    # stash tile accumulator; finalize in batches of 4
                    if af4 is None:
                        af4 = fpool.tile([P, 4, D + 1], F32, tag="af4")
                    slot = len(fin)
                    nc.vector.tensor_copy(out=af4[:, slot, :], in_=A[:])
                    fin.append(t)
                    ti = tj

                    if len(fin) == 4 or ti >= len(chunks):
                        nt = len(fin)
                        dplus = fpool.tile([P, 4], F32, tag="dplus")
                        nc.vector.tensor_scalar_add(
                            dplus[:, :nt], af4[:, :nt, D : D + 1], EPS_DENOM
                        )
                        rr = fpool.tile([P, 4], F32, tag="rr")
                        nc.vector.reciprocal(out=rr[:, :nt], in_=dplus[:, :nt])
                        m = fpool.tile([P, 4, D + 1], F32, tag="m")
                        nc.vector.tensor_tensor(
                            out=m[:, :nt, 0:D], in0=af4[:, :nt, 0:D],
                            in1=rr[:, :nt].to_broadcast([P, nt, D]),
                            op=mybir.AluOpType.mult,
                        )
                        nc.vector.memset(m[:, :nt, D : D + 1], 1.0)
                        o = fpool.tile([P, 4, D], F32, tag="o")
                        for i, tt in enumerate(fin):
                            tp = pfpool.tile([D + 1, P], F32, tag="tp", space="PSUM")
                            nc.tensor.transpose(
                                out=tp[:], in_=m[:, i, :], identity=identf[:]
                            )
                            lhs = fpool.tile([D + 1, P], F32, tag="lhs")
                            nc.vector.tensor_copy(out=lhs[:], in_=tp[:])
                            y = pfpool.tile([P, D], F32, tag="y", space="PSUM")
                            nc.tensor.matmul(
                                out=y[:], lhsT=lhs[:], rhs=wbs[:],
                                start=True, stop=True,
                            )
                            nc.scalar.copy(out=o[:, i, :], in_=y[:])
                        t0 = fin[0]
                        nc.sync.dma_start(
                            out=out[t0 * P : (t0 + nt) * P, :].rearrange(
                                "(a p) d -> p a d", p=P
                            ),
                            in_=o[:, :nt, :],
                        )
                        fin = []
                        af4 = None

    nc.compile()
    return nc


def run2(cfg: Cfg, x, edge_index, z, W, b, alpha, bias_edge, repeat=1):
    from concourse.bass_utils import run_bass_kernel_spmd

    in_maps, caps, TOK = shard_inputs2(cfg, x, z, edge_index)
    wb = np.ascontiguousarray(
        np.concatenate(
            [np.asarray(W, np.float32).T, np.asarray(b, np.float32)[None, :]], axis=0
        )
    )
    for m in in_maps:
        m["wb"] = wb
    nc = build_program2(cfg, float(np.asarray(alpha)), caps, TOK, repeat=repeat)
    core_ids = list(range(cfg.n_cores))
    res = run_bass_kernel_spmd(nc, in_maps, core_ids)
    out = np.concatenate(
        [res.results[k]["out"][: cfg.nodes_per_core] for k in core_ids], axis=0
    )[: cfg.n_nodes]
    return out.astype(np.float32), res


def shard_inputs(cfg: Cfg, x, z, edge_index):
    """Bucket edges by (owner core, col group); deal each row's edges across
    scatter sub-chunks so each scatter instruction has unique rows."""
    D = cfg.d
    row = np.asarray(edge_index[0], dtype=np.int64)
    col = np.asarray(edge_index[1], dtype=np.int64)
    E = row.shape[0]
    core = row // cfg.nodes_per_core
    grp = col // cfg.col_group_size
    bucket = core * cfg.col_groups + grp
    n_bins = cfg.n_cores * cfg.col_groups

    # rank of each edge within its (bucket, row) group
    o = np.lexsort((row, bucket))
    bs, rs = bucket[o], row[o]
    new = np.r_[True, (bs[1:] != bs[:-1]) | (rs[1:] != rs[:-1])]
    gid = np.cumsum(new) - 1
    pos = np.arange(E)
    firstpos = pos[new]
    rank = pos - firstpos[gid]
    maxmult = int(rank.max()) + 1 if E else 1

    # per-(bucket,row) group sizes and within-bucket exclusive cumsum: row r's
    # edges go to chunks (start_r + rank) % n — balanced to +-1 per bucket and
    # unique within each chunk (mult <= n_chunks)
    mult = np.bincount(gid)
    g_bucket = bs[new]
    g_cum = np.concatenate([[0], np.cumsum(mult)[:-1]])
    g_new_bucket = np.r_[True, g_bucket[1:] != g_bucket[:-1]]
    bucket_base = np.maximum.accumulate(np.where(g_new_bucket, g_cum, 0))
    g_start = g_cum - bucket_base
    bucket_counts = np.bincount(bs, minlength=n_bins)
    maxcount = int(bucket_counts.max()) if E else 1

    n_chunks = max(
        (maxcount + cfg.scatter_b - 1) // cfg.scatter_b, maxmult, 1
    )
    while True:
        chunkid = (g_start[gid] + rank) % n_chunks
        cc = np.bincount(bs * n_chunks + chunkid, minlength=n_bins * n_chunks)
        if maxmult <= n_chunks and cc.max() <= cfg.scatter_b:
            break
        n_chunks += 1
    eff = dataclasses.replace(cfg, tokens_per_group=n_chunks * cfg.scatter_b)

    # slot within (bucket, chunk)
    o2 = np.lexsort((chunkid, bs))
    b2, c2 = bs[o2], chunkid[o2]
    new2 = np.r_[True, (b2[1:] != b2[:-1]) | (c2[1:] != c2[:-1])]
    gid2 = np.cumsum(new2) - 1
    firstpos2 = pos[new2]
    rank2 = pos - firstpos2[gid2]
    tokpos = c2 * cfg.scatter_b + rank2

    TG = eff.tokens_per_group
    rl_all = np.full((n_bins, TG), eff.junk_row, np.int16)
    cl_all = np.zeros((n_bins, TG), np.int16)
    edge_sorted = o[o2]  # original edge ids in placement order
    flat = b2 * TG + tokpos
    rl_all.reshape(-1)[flat] = (row[edge_sorted] % cfg.nodes_per_core).astype(np.int16)
    cl_all.reshape(-1)[flat] = (col[edge_sorted] % cfg.col_group_size).astype(np.int16)

    zx = np.concatenate(
        [np.asarray(z, np.float32), np.asarray(x, np.float32)], axis=1
    )
    zx_groups = [
        np.ascontiguousarray(zx[g * cfg.col_group_size : (g + 1) * cfg.col_group_size])
        for g in range(cfg.col_groups)
    ]
    zpad = np.zeros((cfg.n_cores * cfg.nodes_per_core + cfg.acc_rows, D), np.float32)
    zpad[: cfg.n_nodes] = np.asarray(z, np.float32)

    in_maps = []
    for k in range(cfg.n_cores):
        ridx_g = np.stack(
            [_wrap16(rl_all[k * cfg.col_groups + g]) for g in range(cfg.col_groups)]
        )
        cidx_g = np.stack(
            [_wrap16(cl_all[k * cfg.col_groups + g]) for g in range(cfg.col_groups)]
        )
        in_maps.append(
            {
                "zrow": np.ascontiguousarray(
                    zpad[k * cfg.nodes_per_core : k * cfg.nodes_per_core + cfg.acc_rows]
                ),
                **{f"zx{g}": zx_groups[g] for g in range(cfg.col_groups)},
                "ridx": ridx_g,
                "cidx": cidx_g,
            }
        )
    return in_maps, eff


def run(cfg: Cfg, x, edge_index, z, W, b, alpha, bias_edge, trace=False):
    from concourse.bass_utils import run_bass_kernel_spmd

    in_maps, eff = shard_inputs(cfg, x, z, edge_index)
    wb = np.ascontiguousarray(
        np.concatenate(
            [np.asarray(W, np.float32).T, np.asarray(b, np.float32)[None, :]], axis=0
        )
    )
    for m in in_maps:
        m["wb"] = wb
    nc = build_program(eff, float(np.asarray(alpha)))
    core_ids = list(range(eff.n_cores))
    res = run_bass_kernel_spmd(nc, in_maps, core_ids, trace=trace)
    out = np.concatenate(
        [res.results[k]["out"][: eff.nodes_per_core] for k in core_ids], axis=0
    )[: eff.n_nodes]
    return out.astype(np.float32), res


def kernel(**inputs) -> np.ndarray:
    out, _ = run2(
        FULL,
        inputs["x"],
        inputs["edge_index"],
        inputs["z"],
        inputs["W"],
        inputs["b"],
        inputs["alpha"],
        inputs["bias_edge"],
    )
    return out

